# revision 19
# baseline (speedup 1.0000x reference)
"""Trainium2 Bass kernel for the DeltaNet-style block nn_Block_68341519614809.

Sharding: core c = 2*b + g  (b in 0..3 batch, g in 0..1 head-group of 2 heads).
Per core: q/k/v/beta projections on x[b] (d-major via host-transposed x),
depthwise causal conv + SiLU fused on psum blocks (q/k/v staged through DRAM),
chunked delta-rule (WY/UT-transform, chunk=128, triangular inverse via
truncated squaring factorization (I-A)(I+A^2)(I+A^4)(I+A^8) -- residual
~|A|_2^16 ~ 1e-7 for this data, validated end-to-end), per-head RMSNorm,
partial output projection, ReduceScatter over core pairs, residual + LayerNorm
on each core's t-half.  Host gathers the 8 half-outputs.  Matmuls run in
float32r (full PE rate at free-dim>=256, ~13-bit mantissa).
"""
import numpy as np

B, T_FULL, D, H, CONV_K = 4, 2048, 1024, 4, 4
DH = 256          # head dim
DG = 512          # head-group width (2 heads)
EPS = 1e-5
CK = 128          # delta-rule chunk size
TB = 256          # stage-A t-block
N_CORES = 8

_cache = {}


def _build(T=T_FULL, sim_safe=False):
    from contextlib import ExitStack
    import concourse.bacc as bacc
    import concourse.tile as tile
    import concourse.mybir as mybir

    F32 = mybir.dt.float32
    F32R = mybir.dt.float32r
    ALU = mybir.AluOpType
    ACTF = mybir.ActivationFunctionType

    n_tb = T // TB
    n_ck = T // CK
    TH = T // 2  # this core's t-half length
    seg = TB + 4  # f32r matmul free dim must be %4

    nc = bacc.Bacc("TRN2", target_bir_lowering=False, debug=False,
                   num_devices=N_CORES)

    # ---- I/O ----
    xt = nc.dram_tensor("xt", [D, T + 4], F32R, kind="ExternalInput")  # 4 zero cols + x[b].T
    xres = nc.dram_tensor("xres", [TH, D], F32, kind="ExternalInput")    # x[b, t-half]
    wq = nc.dram_tensor("wq", [D, DG], F32R, kind="ExternalInput")
    wk = nc.dram_tensor("wk", [D, DG], F32R, kind="ExternalInput")
    wv = nc.dram_tensor("wv", [D, DG], F32R, kind="ExternalInput")
    wb2 = nc.dram_tensor("wb2", [D, 2], F32R, kind="ExternalInput")
    cqT = nc.dram_tensor("cqT", [DG, CONV_K], F32, kind="ExternalInput")
    ckT = nc.dram_tensor("ckT", [DG, CONV_K], F32, kind="ExternalInput")
    cvT = nc.dram_tensor("cvT", [DG, CONV_K], F32, kind="ExternalInput")
    grmsb = nc.dram_tensor("grmsb", [128, DH], F32, kind="ExternalInput")  # bcast
    wo = nc.dram_tensor("wo", [DG, D], F32R, kind="ExternalInput")
    lng = nc.dram_tensor("lng", [128, D], F32, kind="ExternalInput")     # broadcast
    lnb = nc.dram_tensor("lnb", [128, D], F32, kind="ExternalInput")     # broadcast
    ident_in = nc.dram_tensor("ident", [128, 128], F32R, kind="ExternalInput")
    ones_in = nc.dram_tensor("ones", [128, 128], F32R, kind="ExternalInput")
    zs_in = nc.dram_tensor("zs", [128, DH], F32R, kind="ExternalInput")
    mlo_in = nc.dram_tensor("mlo", [128, 128], F32, kind="ExternalInput")   # -(j<i)
    mup_in = nc.dram_tensor("mup", [128, 128], F32, kind="ExternalInput")   # -(j>i)
    mui_in = nc.dram_tensor("mui", [128, 128], F32, kind="ExternalInput")   # (j>=i)
    y_out = nc.dram_tensor("y_out", [TH, D], F32, kind="ExternalOutput")

    with tile.TileContext(nc) as tc, ExitStack() as top:
        top.enter_context(nc.allow_low_precision(
            reason="float32r is full-width fp32 storage; PE rounds on ingest"))
        const = top.enter_context(tc.tile_pool(name="const", bufs=1))
        # PSUM: 4 tags x 2 bufs = 8 banks exactly
        psum = top.enter_context(tc.tile_pool(name="psum", bufs=2, space="PSUM"))
        dram = top.enter_context(tc.tile_pool(name="dram", bufs=1, space="DRAM"))

        def ps_pay():
            return psum.tile([128, 512], F32, tag="pay", name="pay")

        def ps_pg():
            return psum.tile([128, 128], F32, tag="pg", name="pg")

        def ps_med():
            return psum.tile([128, 512], F32, tag="pmed", name="pmed")

        def ps_small():
            return psum.tile([128, 384], F32, tag="psmall", name="psmall")

        # ---- constants ----
        IDENT = const.tile([128, 128], F32R, tag="ident", name="ident")
        nc.sync.dma_start(IDENT[:], ident_in[:])
        MLO = const.tile([128, 128], F32, tag="mlo", name="mlo")
        nc.sync.dma_start(MLO[:], mlo_in[:])
        MUP = const.tile([128, 128], F32, tag="mup", name="mup")
        nc.sync.dma_start(MUP[:], mup_in[:])
        MUI = const.tile([128, 128], F32, tag="mui", name="mui")
        nc.sync.dma_start(MUI[:], mui_in[:])
        ONESR = const.tile([128, 128], F32R, tag="onesr", name="onesr")
        nc.sync.dma_start(ONESR[:], ones_in[:])
        EPS1 = const.tile([128, 1], F32, tag="eps1", name="eps1")
        nc.gpsimd.memset(EPS1[:], 1e-6)
        EPSL = const.tile([128, 1], F32, tag="epsl", name="epsl")
        nc.gpsimd.memset(EPSL[:], EPS)
        GRMSB = const.tile([128, DH], F32, tag="grmsb", name="grmsb")
        nc.sync.dma_start(GRMSB[:], grmsb[:])
        CW = {}
        for nm, cw in (("q", cqT), ("k", ckT), ("v", cvT)):
            CW[nm] = const.tile([128, 16], F32, tag=f"cw{nm}", name=f"cw{nm}")
            nc.sync.dma_start(CW[nm][:].rearrange("p (dt j) -> p dt j", dt=4),
                              cw[:].rearrange("(dt p) j -> p dt j", p=128))
        WB2 = const.tile([128, 16], F32R, tag="wb2", name="wb2")
        nc.sync.dma_start(WB2[:].rearrange("p (k j) -> p k j", k=8),
                          wb2[:].rearrange("(k p) j -> p k j", p=128))

        # beta rows stay in SBUF; q/k/v stream through DRAM
        BT = [const.tile([1, T], F32, tag=f"BT{h}", name=f"BT{h}") for h in range(2)]
        qdr = dram.tile([DG, T], F32R, tag="qdr", name="qdr")
        kdr = dram.tile([DG, T], F32R, tag="kdr", name="kdr")
        vdr = dram.tile([DG, T], F32R, tag="vdr", name="vdr")
        TGT = {"q": qdr, "k": kdr, "v": vdr}

        # ================= stage A: projections + conv + silu =================
        with ExitStack() as sa:
            wpool = sa.enter_context(tc.tile_pool(name="wpool", bufs=1))
            xbp = sa.enter_context(tc.tile_pool(name="xbp", bufs=2))
            cvp = sa.enter_context(tc.tile_pool(name="cvp", bufs=3))

            WQT = [wpool.tile([128, DG], F32R, tag=f"wq{k}", name=f"wq{k}")
                   for k in range(8)]
            WKT = [wpool.tile([128, DG], F32R, tag=f"wk{k}", name=f"wk{k}")
                   for k in range(8)]
            WVT = [wpool.tile([128, DG], F32R, tag=f"wv{k}", name=f"wv{k}")
                   for k in range(8)]
            for k in range(8):
                nc.sync.dma_start(WQT[k][:], wq[k * 128:(k + 1) * 128, :])
                nc.sync.dma_start(WKT[k][:], wk[k * 128:(k + 1) * 128, :])
                nc.sync.dma_start(WVT[k][:], wv[k * 128:(k + 1) * 128, :])
            WT = {"q": WQT, "k": WKT, "v": WVT}

            for tb in range(n_tb):
                t0 = tb * TB
                xb = xbp.tile([128, 8 * seg], F32R, tag="xb", name="xb")
                for k in range(8):
                    nc.sync.dma_start(xb[:, k * seg:(k + 1) * seg],
                                      xt[k * 128:(k + 1) * 128, t0:t0 + seg])
                # beta rows (one [1,TB] psum per head: matmul base-partition rule)
                for h in range(2):
                    psb = ps_small()
                    for k in range(8):
                        nc.tensor.matmul(psb[0:1, 0:TB],
                                         WB2[:, k * 2 + h:k * 2 + h + 1],
                                         xb[:, k * seg + 4:(k + 1) * seg],
                                         start=(k == 0), stop=(k == 7))
                    nc.scalar.activation(BT[h][0:1, t0:t0 + TB], psb[0:1, 0:TB],
                                         ACTF.Sigmoid)

                for nm in ("q", "k", "v"):
                    for j in range(4):
                        ps = ps_pay()
                        for k in range(8):
                            nc.tensor.matmul(ps[:, 0:seg],
                                             WT[nm][k][:, j * 128:(j + 1) * 128],
                                             xb[:, k * seg:(k + 1) * seg],
                                             start=(k == 0), stop=(k == 7))
                        cw = CW[nm]
                        c0 = cw[:, j * 4 + 0:j * 4 + 1]
                        c1 = cw[:, j * 4 + 1:j * 4 + 2]
                        c2 = cw[:, j * 4 + 2:j * 4 + 3]
                        c3 = cw[:, j * 4 + 3:j * 4 + 4]
                        # taps: conv[t] = sum_i cw[i]*pre[t-3+i]; ps col (t-t0+4)
                        m0 = cvp.tile([128, TB], F32, tag="m0", name="m0")
                        nc.scalar.activation(m0[:], ps[:, 1:TB + 1], ACTF.Copy,
                                             scale=c0)
                        m1 = cvp.tile([128, TB], F32, tag="m1", name="m1")
                        nc.scalar.activation(m1[:], ps[:, 2:TB + 2], ACTF.Copy,
                                             scale=c1)
                        s2 = cvp.tile([128, TB], F32, tag="s2", name="s2")
                        nc.vector.scalar_tensor_tensor(s2[:], ps[:, 3:TB + 3], c2,
                                                       m0[:], ALU.mult, ALU.add)
                        s3 = cvp.tile([128, TB], F32, tag="s3", name="s3")
                        nc.vector.scalar_tensor_tensor(s3[:], ps[:, 4:TB + 4], c3,
                                                       m1[:], ALU.mult, ALU.add)
                        cv_ = cvp.tile([128, TB], F32, tag="cv", name="cv")
                        nc.gpsimd.tensor_tensor(cv_[:], s2[:], s3[:], ALU.add)
                        st = cvp.tile([128, TB], F32R, tag="st", name="st")
                        if sim_safe:  # CoreSim lacks Silu; HW has it
                            sg = cvp.tile([128, TB], F32, tag="sg", name="sg")
                            nc.scalar.activation(sg[:], cv_[:], ACTF.Sigmoid)
                            nc.gpsimd.tensor_tensor(st[:], cv_[:], sg[:], ALU.mult)
                        else:
                            nc.scalar.activation(st[:], cv_[:], ACTF.Silu)
                        nc.sync.dma_start(
                            TGT[nm][j * 128:(j + 1) * 128, t0:t0 + TB], st[:])

        # ================= chunk stage: delta rule =================
        ckx = top.enter_context(ExitStack())
        work = ckx.enter_context(tc.tile_pool(name="work", bufs=2))
        spool = ckx.enter_context(tc.tile_pool(name="spool", bufs=2))
        ohp = ckx.enter_context(tc.tile_pool(name="ohp", bufs=2))
        wop = ckx.enter_context(tc.tile_pool(name="wop", bufs=1))
        qkt = ckx.enter_context(tc.tile_pool(name="qkt", bufs=3))

        WO = [wop.tile([128, D], F32R, tag=f"wo{k}", name=f"wo{k}") for k in range(4)]
        for k in range(4):
            nc.sync.dma_start(WO[k][:], wo[k * 128:(k + 1) * 128, :])

        ydr = dram.tile([T, D], F32, tag="ydr", name="ydr")

        S = {}
        for h in range(2):
            S[h] = [spool.tile([128, DH], F32R, tag=f"S{h}{i}", name=f"S{h}{i}")
                    for i in range(2)]
            for i in range(2):
                nc.sync.dma_start(S[h][i][:], zs_in[:])

        def chunk_step(h, c):
            cc = slice(c * CK, (c + 1) * CK)
            r0 = 256 * h
            # stream q/k/v chunk (d-major, two d-tiles side by side)
            qc = qkt.tile([128, 256], F32R, tag="qc", name="qc")
            kc = qkt.tile([128, 256], F32R, tag="kc", name="kc")
            vc = qkt.tile([128, 256], F32R, tag="vc", name="vc")
            for i in range(2):
                sl = slice(i * 128, (i + 1) * 128)
                rr = slice(r0 + i * 128, r0 + (i + 1) * 128)
                nc.sync.dma_start(qc[:, sl], qdr[rr, cc])
                nc.sync.dma_start(kc[:, sl], kdr[rr, cc])
                nc.sync.dma_start(vc[:, sl], vdr[rr, cc])
            brow = BT[h][0:1, cc]

            # -- l2 norm sums: SQ layout [q-d0 | k-d0 | q-d1 | k-d1] --
            SQ = work.tile([128, 512], F32R, tag="SQ", name="SQ")
            for i in range(2):
                nc.scalar.activation(SQ[:, i * 256:i * 256 + 128],
                                     qc[:, i * 128:(i + 1) * 128], ACTF.Square)
                nc.scalar.activation(SQ[:, i * 256 + 128:(i + 1) * 256],
                                     kc[:, i * 128:(i + 1) * 128], ACTF.Square)
            psn = ps_small()
            for i in range(2):
                nc.tensor.matmul(psn[0:1, 0:256], ONESR[:, 0:1],
                                 SQ[:, i * 256:(i + 1) * 256],
                                 start=(i == 0), stop=(i == 1))
            # rows: rq | rk | rkb
            sqr = work.tile([1, 256], F32, tag="sqr", name="sqr")
            nc.scalar.activation(sqr[:], psn[0:1, 0:256], ACTF.Sqrt,
                                 bias=EPS1[0:1, 0:1])
            R3 = work.tile([1, 384], F32R, tag="R3", name="R3")
            nc.vector.reciprocal(R3[0:1, 0:256], sqr[:])
            nc.vector.tensor_tensor(R3[0:1, 256:384], R3[0:1, 128:256], brow,
                                    ALU.mult)
            # beta column
            psbc = ps_small()
            nc.tensor.transpose(psbc[0:128, 0:1], brow, MUI[0:1, 0:1])
            bcol = work.tile([128, 1], F32, tag="bcol", name="bcol")
            nc.scalar.copy(bcol[:], psbc[0:128, 0:1])
            # broadcast rows across partitions: [BRq | BRk | BRkb]
            psbr = ps_small()
            nc.tensor.matmul(psbr[0:128, 0:384], ONESR[0:1, :], R3[:],
                             start=True, stop=True)
            # normalized d-major tiles (KbT = beta * normalized k)
            QhT = work.tile([128, 256], F32R, tag="QhT", name="QhT")
            KhT = work.tile([128, 256], F32R, tag="KhT", name="KhT")
            KbT = work.tile([128, 256], F32R, tag="KbT", name="KbT")
            for i in range(2):
                sl = slice(i * 128, (i + 1) * 128)
                nc.vector.tensor_tensor(QhT[:, sl], qc[:, sl], psbr[:, 0:128],
                                        ALU.mult)
                nc.vector.tensor_tensor(KhT[:, sl], kc[:, sl], psbr[:, 128:256],
                                        ALU.mult)
                nc.vector.tensor_tensor(KbT[:, sl], kc[:, sl], psbr[:, 256:384],
                                        ALU.mult)

            # -- Gram + masks: N = -tril(Kb K^T,-1), NT = -triu(K Kb^T,1) --
            psg = ps_pg()
            for i in range(2):
                sl = slice(i * 128, (i + 1) * 128)
                nc.tensor.matmul(psg[:], KbT[:, sl], KhT[:, sl],
                                 start=(i == 0), stop=(i == 1))
            N = work.tile([128, 128], F32R, tag="N", name="N")
            nc.vector.tensor_tensor(N[:], psg[:], MLO[:], ALU.mult)
            psgt = ps_pg()
            for i in range(2):
                sl = slice(i * 128, (i + 1) * 128)
                nc.tensor.matmul(psgt[:], KhT[:, sl], KbT[:, sl],
                                 start=(i == 0), stop=(i == 1))
            NT = work.tile([128, 128], F32R, tag="NT", name="NT")
            nc.vector.tensor_tensor(NT[:], psgt[:], MUP[:], ALU.mult)

            # -- truncated inverse: T'^T=(I-A^T)(I+(A^T)^2)(I+(A^T)^4)(I+(A^T)^8)
            pp = ps_pg()
            nc.tensor.matmul(pp[:], NT[:], N[:], start=True, stop=True)   # A^2
            P2 = work.tile([128, 128], F32R, tag="P2", name="P2")
            nc.scalar.copy(P2[:], pp[:])
            ppt = ps_pg()
            nc.tensor.matmul(ppt[:], N[:], NT[:], start=True, stop=True)  # (A^T)^2
            P2T = work.tile([128, 128], F32R, tag="P2T", name="P2T")
            nc.vector.tensor_copy(P2T[:], ppt[:])
            pp4 = ps_pg()
            nc.tensor.matmul(pp4[:], P2T[:], P2[:], start=True, stop=True)  # A^4
            P4 = work.tile([128, 128], F32R, tag="P4", name="P4")
            nc.scalar.copy(P4[:], pp4[:])
            pp4t = ps_pg()
            nc.tensor.matmul(pp4t[:], P2[:], P2T[:], start=True, stop=True)
            P4T = work.tile([128, 128], F32R, tag="P4T", name="P4T")
            nc.vector.tensor_copy(P4T[:], pp4t[:])
            pp8t = ps_pg()
            nc.tensor.matmul(pp8t[:], P4[:], P4T[:], start=True, stop=True)
            R = work.tile([128, 128], F32R, tag="Rch", name="Rch")
            nc.vector.tensor_tensor(R[:], IDENT[:], pp8t[:], ALU.add)
            for P in (P4, P2):
                pst = ps_pg()
                nc.tensor.matmul(pst[:], P[:], R[:], start=True, stop=True)
                R2 = work.tile([128, 128], F32R, tag="Rch", name="Rch")
                nc.vector.tensor_tensor(R2[:], R[:], pst[:], ALU.add)
                R = R2
            pst = ps_pg()
            nc.tensor.matmul(pst[:], N[:], R[:], start=True, stop=True)
            TT = work.tile([128, 128], F32R, tag="TT", name="TT")
            nc.vector.scalar_tensor_tensor(TT[:], R[:], 1.0, pst[:], ALU.mult,
                                           ALU.add)

            # -- W2b = beta*V - (beta K) S  (t-major; V-transpose + KS share bank)
            vw = ps_med()
            for i in range(2):
                nc.tensor.transpose(vw[:, i * 128:(i + 1) * 128].bitcast(F32R),
                                    vc[:, i * 128:(i + 1) * 128], IDENT[:])
            Vtb = work.tile([128, 256], F32R, tag="Vtb", name="Vtb")
            nc.scalar.activation(Vtb[:], vw[:, 0:256], ACTF.Copy, scale=bcol[:])
            for i in range(2):
                nc.tensor.matmul(vw[:, 256:512], KbT[:, i * 128:(i + 1) * 128],
                                 S[h][i][:], start=(i == 0), stop=(i == 1))
            W2b = work.tile([128, 256], F32R, tag="W2b", name="W2b")
            nc.vector.tensor_tensor(W2b[:], Vtb[:], vw[:, 256:512], ALU.subtract)

            # -- U = T' W2b --
            psu = ps_med()
            nc.tensor.matmul(psu[:, 0:256], TT[:], W2b[:], start=True, stop=True)
            U = work.tile([128, 256], F32R, tag="U", name="U")
            nc.scalar.copy(U[:], psu[:, 0:256])

            # -- MT = triu(K Q^T) incl diag --
            psmt = ps_pg()
            for i in range(2):
                sl = slice(i * 128, (i + 1) * 128)
                nc.tensor.matmul(psmt[:], KhT[:, sl], QhT[:, sl],
                                 start=(i == 0), stop=(i == 1))
            MT = work.tile([128, 128], F32R, tag="MT", name="MT")
            nc.vector.tensor_tensor(MT[:], psmt[:], MUI[:], ALU.mult)

            # -- O (t-major) = Q S + M U;  RMS stats via ACT accum --
            pso = ps_med()
            nc.tensor.matmul(pso[:, 0:256], QhT[:, 0:128], S[h][0][:],
                             start=True, stop=False)
            nc.tensor.matmul(pso[:, 0:256], QhT[:, 128:256], S[h][1][:],
                             start=False, stop=False)
            nc.tensor.matmul(pso[:, 0:256], MT[:], U[:], start=False, stop=True)
            sqw = work.tile([128, 256], F32, tag="sqw", name="sqw")
            sso = work.tile([128, 1], F32, tag="sso", name="sso")
            nc.scalar.activation(sqw[:], pso[:, 0:256], ACTF.Square,
                                 accum_out=sso[:])
            sdo = work.tile([128, 1], F32, tag="sdo", name="sdo")
            nc.scalar.activation(sdo[:], sso[:], ACTF.Sqrt, bias=EPSL[:],
                                 scale=1.0 / DH)
            rco = work.tile([128, 1], F32, tag="rco", name="rco")
            nc.vector.reciprocal(rco[:], sdo[:])
            Ohn = work.tile([128, 256], F32R, tag="Ohn", name="Ohn")
            nc.vector.scalar_tensor_tensor(Ohn[:], pso[:, 0:256], rco[:], GRMSB[:],
                                           ALU.mult, ALU.mult)
            # transpose to d-major for the output projection
            for i in range(2):
                nc.tensor.transpose(
                    pso[:, 256 + i * 128:256 + (i + 1) * 128].bitcast(F32R),
                    Ohn[:, i * 128:(i + 1) * 128], IDENT[:])
            OhT = ohp.tile([128, 256], F32R, tag=f"OhT{h}", name=f"OhT{h}")
            nc.scalar.copy(OhT[:], pso[:, 256:512])

            # -- S += K^T U  (transposes then the two dS matmuls share the bank)
            ktds = ps_med()
            for i in range(2):
                nc.tensor.transpose(ktds[:, i * 128:(i + 1) * 128].bitcast(F32R),
                                    KhT[:, i * 128:(i + 1) * 128], IDENT[:])
            Kh = work.tile([128, 256], F32R, tag="Kh", name="Kh")
            nc.scalar.copy(Kh[:], ktds[:, 0:256])
            for i in range(2):
                reg = slice(256, 512) if i == 0 else slice(0, 256)
                nc.tensor.matmul(ktds[:, reg], Kh[:, i * 128:(i + 1) * 128], U[:],
                                 start=True, stop=True)
                Snew = spool.tile([128, DH], F32R, tag=f"S{h}{i}", name=f"S{h}{i}")
                nc.vector.tensor_tensor(Snew[:], S[h][i][:], ktds[:, reg], ALU.add)
                S[h][i] = Snew
            return OhT

        for c in range(n_ck):
            oht = [chunk_step(h, c) for h in range(2)]
            # -- partial y = o @ Wo for this chunk --
            for n in range(2):
                psy = ps_pay()
                for kk in range(4):
                    h, i = kk // 2, kk % 2
                    nc.tensor.matmul(psy[:], oht[h][:, i * 128:(i + 1) * 128],
                                     WO[kk][:, n * 512:(n + 1) * 512],
                                     start=(kk == 0), stop=(kk == 3))
                ysb = work.tile([128, 512], F32, tag="ysb", name="ysb")
                if n == 0:
                    nc.scalar.copy(ysb[:], psy[:])
                else:
                    nc.vector.tensor_copy(ysb[:], psy[:])
                nc.sync.dma_start(ydr[c * CK:(c + 1) * CK, n * 512:(n + 1) * 512],
                                  ysb[:])

        ckx.close()

        # ======== ReduceScatter over pairs + residual + LN on our t-half ========
        yhalf = dram.tile([TH, D], F32, tag="yhalf", name="yhalf")
        nc.gpsimd.collective_compute(
            "ReduceScatter", ALU.add,
            replica_groups=[[0, 1], [2, 3], [4, 5], [6, 7]],
            ins=[ydr.opt()], outs=[yhalf.opt()],
        )

        LNG = const.tile([128, D], F32, tag="lng", name="lng")
        nc.sync.dma_start(LNG[:], lng[:])
        LNB = const.tile([128, D], F32, tag="lnb", name="lnb")
        nc.sync.dma_start(LNB[:], lnb[:])

        lnp = top.enter_context(tc.tile_pool(name="lnp", bufs=2))
        for r in range(TH // 128):
            rs = slice(r * 128, (r + 1) * 128)
            yr_in = lnp.tile([128, D], F32, tag="yr_in", name="yr_in")
            nc.sync.dma_start(yr_in[:], yhalf[rs, :])
            xr = lnp.tile([128, D], F32, tag="xr", name="xr")
            nc.sync.dma_start(xr[:], xres[rs, :])
            yr = lnp.tile([128, D], F32, tag="yr", name="yr")
            nc.gpsimd.tensor_tensor(yr[:], yr_in[:], xr[:], ALU.add)
            # mean
            waste = lnp.tile([128, D], F32, tag="waste", name="waste", bufs=1)
            srow = lnp.tile([128, 1], F32, tag="srow", name="srow")
            nc.scalar.activation(waste[:], yr[:], ACTF.Identity, accum_out=srow[:])
            mneg = lnp.tile([128, 1], F32, tag="mneg", name="mneg")
            nc.scalar.mul(mneg[:], srow[:], -1.0 / D)
            yc = lnp.tile([128, D], F32, tag="yc", name="yc")
            nc.scalar.activation(yc[:], yr[:], ACTF.Identity, bias=mneg[:])
            ssq = lnp.tile([128, 1], F32, tag="ssq", name="ssq")
            nc.scalar.activation(waste[:], yc[:], ACTF.Square, accum_out=ssq[:])
            sd = lnp.tile([128, 1], F32, tag="sd", name="sd")
            nc.scalar.activation(sd[:], ssq[:], ACTF.Sqrt, bias=EPSL[:],
                                 scale=1.0 / D)
            rcol = lnp.tile([128, 1], F32, tag="rcol", name="rcol")
            nc.vector.reciprocal(rcol[:], sd[:])
            yn = lnp.tile([128, D], F32, tag="yn", name="yn")
            nc.vector.scalar_tensor_tensor(yn[:], yc[:], rcol[:], LNG[:],
                                           ALU.mult, ALU.mult)
            yfin = lnp.tile([128, D], F32, tag="yfin", name="yfin")
            nc.vector.tensor_tensor(yfin[:], yn[:], LNB[:], ALU.add)
            nc.sync.dma_start(y_out[rs, :], yfin[:])

    nc.compile()
    return nc


def _shard(inputs, T=T_FULL):
    x = np.ascontiguousarray(np.asarray(inputs["x"]), dtype=np.float32)
    f32 = lambda a: np.ascontiguousarray(np.asarray(a), dtype=np.float32)
    Wq, Wk, Wv = inputs["Wq"], inputs["Wk"], inputs["Wv"]
    Wb, Wo = inputs["Wb"], inputs["Wo"]
    cq, ck, cv = inputs["conv_q"], inputs["conv_k"], inputs["conv_v"]
    g_rms, ln_g, ln_b = inputs["g_rms"], inputs["ln_g"], inputs["ln_b"]
    TH = T // 2

    ident = np.eye(128, dtype=np.float32)
    ii, jj = np.indices((128, 128))
    mlo = -(jj < ii).astype(np.float32)
    mup = -(jj > ii).astype(np.float32)
    mui = (jj >= ii).astype(np.float32)

    in_maps = []
    for c in range(N_CORES):
        b, g = c // 2, c % 2
        gs = slice(g * DG, (g + 1) * DG)
        in_maps.append({
            "xt": f32(np.concatenate([np.zeros((D, 4), np.float32),
                                      x[b, :T].T], axis=1)),
            "xres": f32(x[b, :T][g * TH:(g + 1) * TH]),
            "wq": f32(np.asarray(Wq)[:, gs]), "wk": f32(np.asarray(Wk)[:, gs]),
            "wv": f32(np.asarray(Wv)[:, gs]),
            "wb2": f32(np.asarray(Wb)[:, 2 * g:2 * g + 2]),
            "cqT": f32(np.asarray(cq)[:, gs].T), "ckT": f32(np.asarray(ck)[:, gs].T),
            "cvT": f32(np.asarray(cv)[:, gs].T),
            "grmsb": f32(np.tile(np.asarray(g_rms)[None, :], (128, 1))),
            "wo": f32(np.asarray(Wo)[gs, :]),
            "lng": f32(np.tile(np.asarray(ln_g)[None, :], (128, 1))),
            "lnb": f32(np.tile(np.asarray(ln_b)[None, :], (128, 1))),
            "ident": ident, "ones": np.ones((128, 128), np.float32),
            "zs": np.zeros((128, DH), np.float32),
            "mlo": mlo, "mup": mup, "mui": mui,
        })
    return in_maps


def kernel(**inputs):
    from concourse.bass_utils import run_bass_kernel_spmd
    T = T_FULL
    if "nc" not in _cache:
        _cache["nc"] = _build(T)
    nc = _cache["nc"]
    in_maps = _shard(inputs, T)
    res = run_bass_kernel_spmd(nc, in_maps, core_ids=list(range(N_CORES)))
    out = np.empty((B, T, D), dtype=np.float32)
    TH = T // 2
    for c in range(N_CORES):
        b, g = c // 2, c % 2
        out[b, g * TH:(g + 1) * TH] = res.results[c]["y_out"]
    return out


# revision 22
# speedup vs baseline: 1.1743x; 1.1743x over previous
"""Trainium2 Bass kernel for the DeltaNet-style block nn_Block_68341519614809.

Sharding: core c = 2*b + g  (b in 0..3 batch, g in 0..1 head-group of 2 heads).
Per core: q/k/v/beta projections on x[b] (d-major via host-transposed x),
depthwise causal conv + SiLU fused on psum blocks (q/k/v staged through DRAM),
chunked delta-rule (WY/UT-transform, chunk=128, triangular inverse via
truncated squaring factorization (I-A)(I+A^2)(I+A^4)(I+A^8) -- residual
~|A|_2^16 ~ 1e-7 for this data, validated end-to-end), per-head RMSNorm,
partial output projection, ReduceScatter over core pairs, residual + LayerNorm
on each core's t-half.  Host gathers the 8 half-outputs.  Matmuls run in
float32r (full PE rate at free-dim>=256, ~13-bit mantissa).
"""
import numpy as np

B, T_FULL, D, H, CONV_K = 4, 2048, 1024, 4, 4
DH = 256          # head dim
DG = 512          # head-group width (2 heads)
EPS = 1e-5
CK = 128          # delta-rule chunk size
TB = 256          # stage-A t-block
N_CORES = 8

_cache = {}


def _rows_idx(T):
    """Global t-rows owned by head-group g under the blocked ReduceScatter."""
    n_ck = T // CK
    n_rs = 4 if n_ck % 4 == 0 else 2
    TBL = T // n_rs
    HALF = TBL // 2
    import numpy as _np
    out = {}
    for g in range(2):
        out[g] = _np.concatenate(
            [_np.arange(rb * TBL + g * HALF, rb * TBL + g * HALF + HALF)
             for rb in range(n_rs)])
    return out


def _build(T=T_FULL, sim_safe=False):
    from contextlib import ExitStack
    import concourse.bacc as bacc
    import concourse.tile as tile
    import concourse.mybir as mybir

    F32 = mybir.dt.float32
    F32R = mybir.dt.float32r
    ALU = mybir.AluOpType
    ACTF = mybir.ActivationFunctionType

    n_tb = T // TB
    n_ck = T // CK
    TH = T // 2  # this core's t-half length
    seg = TB + 4  # f32r matmul free dim must be %4

    nc = bacc.Bacc("TRN2", target_bir_lowering=False, debug=False,
                   num_devices=N_CORES)

    # ---- I/O ----
    xt = nc.dram_tensor("xt", [D, T + 4], F32R, kind="ExternalInput")  # 4 zero cols + x[b].T
    xres = nc.dram_tensor("xres", [TH, D], F32, kind="ExternalInput")    # x[b, t-half]
    wq = nc.dram_tensor("wq", [D, DG], F32R, kind="ExternalInput")
    wk = nc.dram_tensor("wk", [D, DG], F32R, kind="ExternalInput")
    wv = nc.dram_tensor("wv", [D, DG], F32R, kind="ExternalInput")
    wb2 = nc.dram_tensor("wb2", [D, 2], F32R, kind="ExternalInput")
    cqT = nc.dram_tensor("cqT", [DG, CONV_K], F32, kind="ExternalInput")
    ckT = nc.dram_tensor("ckT", [DG, CONV_K], F32, kind="ExternalInput")
    cvT = nc.dram_tensor("cvT", [DG, CONV_K], F32, kind="ExternalInput")
    grmsb = nc.dram_tensor("grmsb", [128, DH], F32, kind="ExternalInput")  # bcast
    wo = nc.dram_tensor("wo", [DG, D], F32R, kind="ExternalInput")
    lng = nc.dram_tensor("lng", [128, D], F32, kind="ExternalInput")     # broadcast
    lnb = nc.dram_tensor("lnb", [128, D], F32, kind="ExternalInput")     # broadcast
    ident_in = nc.dram_tensor("ident", [128, 128], F32R, kind="ExternalInput")
    ones_in = nc.dram_tensor("ones", [128, 128], F32R, kind="ExternalInput")
    zs_in = nc.dram_tensor("zs", [128, DH], F32R, kind="ExternalInput")
    mlo_in = nc.dram_tensor("mlo", [128, 128], F32, kind="ExternalInput")   # -(j<i)
    mup_in = nc.dram_tensor("mup", [128, 128], F32, kind="ExternalInput")   # -(j>i)
    mui_in = nc.dram_tensor("mui", [128, 128], F32, kind="ExternalInput")   # (j>=i)
    y_out = nc.dram_tensor("y_out", [TH, D], F32, kind="ExternalOutput")

    with tile.TileContext(nc) as tc, ExitStack() as top:
        top.enter_context(nc.allow_low_precision(
            reason="float32r is full-width fp32 storage; PE rounds on ingest"))
        const = top.enter_context(tc.tile_pool(name="const", bufs=1))
        # PSUM: 4 tags x 2 bufs = 8 banks exactly
        psum = top.enter_context(tc.tile_pool(name="psum", bufs=2, space="PSUM"))
        dram = top.enter_context(tc.tile_pool(name="dram", bufs=1, space="DRAM"))

        def ps_pay():
            return psum.tile([128, 512], F32, tag="pay", name="pay")

        def ps_pg():
            return psum.tile([128, 128], F32, tag="pg", name="pg")

        def ps_med():
            return psum.tile([128, 512], F32, tag="pmed", name="pmed")

        def ps_small():
            return psum.tile([128, 384], F32, tag="psmall", name="psmall")

        # ---- constants ----
        IDENT = const.tile([128, 128], F32R, tag="ident", name="ident")
        nc.sync.dma_start(IDENT[:], ident_in[:])
        MLO = const.tile([128, 128], F32, tag="mlo", name="mlo")
        nc.sync.dma_start(MLO[:], mlo_in[:])
        MUP = const.tile([128, 128], F32, tag="mup", name="mup")
        nc.sync.dma_start(MUP[:], mup_in[:])
        MUI = const.tile([128, 128], F32, tag="mui", name="mui")
        nc.sync.dma_start(MUI[:], mui_in[:])
        ONESR = const.tile([128, 128], F32R, tag="onesr", name="onesr")
        nc.sync.dma_start(ONESR[:], ones_in[:])
        EPS1 = const.tile([128, 1], F32, tag="eps1", name="eps1")
        nc.gpsimd.memset(EPS1[:], 1e-6)
        EPSL = const.tile([128, 1], F32, tag="epsl", name="epsl")
        nc.gpsimd.memset(EPSL[:], EPS)
        GRMSB = const.tile([128, DH], F32, tag="grmsb", name="grmsb")
        nc.sync.dma_start(GRMSB[:], grmsb[:])
        CW = {}
        for nm, cw in (("q", cqT), ("k", ckT), ("v", cvT)):
            CW[nm] = const.tile([128, 16], F32, tag=f"cw{nm}", name=f"cw{nm}")
            nc.sync.dma_start(CW[nm][:].rearrange("p (dt j) -> p dt j", dt=4),
                              cw[:].rearrange("(dt p) j -> p dt j", p=128))
        WB2 = const.tile([128, 16], F32R, tag="wb2", name="wb2")
        nc.sync.dma_start(WB2[:].rearrange("p (k j) -> p k j", k=8),
                          wb2[:].rearrange("(k p) j -> p k j", p=128))

        # beta rows stay in SBUF; q/k/v stream through DRAM
        BT = [const.tile([1, T], F32, tag=f"BT{h}", name=f"BT{h}") for h in range(2)]
        qdr = dram.tile([DG, T], F32R, tag="qdr", name="qdr")
        kdr = dram.tile([DG, T], F32R, tag="kdr", name="kdr")
        vdr = dram.tile([DG, T], F32R, tag="vdr", name="vdr")
        TGT = {"q": qdr, "k": kdr, "v": vdr}

        # ================= stage A: projections + conv + silu =================
        with ExitStack() as sa:
            wpool = sa.enter_context(tc.tile_pool(name="wpool", bufs=1))
            xbp = sa.enter_context(tc.tile_pool(name="xbp", bufs=2))
            cvp = sa.enter_context(tc.tile_pool(name="cvp", bufs=3))

            WT = {}
            for nm, wsrc in (("q", wq), ("k", wk), ("v", wv)):
                wt = wpool.tile([128, 8 * DG], F32R, tag=f"w{nm}", name=f"w{nm}")
                nc.sync.dma_start(wt[:].rearrange("p (k d) -> p k d", k=8),
                                  wsrc[:].rearrange("(k p) d -> p k d", p=128))
                WT[nm] = wt

            for tb in range(n_tb):
                t0 = tb * TB
                xb = xbp.tile([128, 8 * seg], F32R, tag="xb", name="xb")
                nc.sync.dma_start(
                    xb[:].rearrange("p (k t) -> p k t", k=8),
                    xt[:].rearrange("(k p) t -> p k t", p=128)[:, :, t0:t0 + seg])
                # beta rows (one [1,TB] psum per head: matmul base-partition rule)
                for h in range(2):
                    psb = ps_small()
                    for k in range(8):
                        nc.tensor.matmul(psb[0:1, 0:TB],
                                         WB2[:, k * 2 + h:k * 2 + h + 1],
                                         xb[:, k * seg + 4:(k + 1) * seg],
                                         start=(k == 0), stop=(k == 7))
                    nc.scalar.activation(BT[h][0:1, t0:t0 + TB], psb[0:1, 0:TB],
                                         ACTF.Sigmoid)

                for nm in ("q", "k", "v"):
                    for j in range(4):
                        ps = ps_pay()
                        for k in range(8):
                            nc.tensor.matmul(ps[:, 0:seg],
                                             WT[nm][:, k * DG + j * 128:
                                                        k * DG + (j + 1) * 128],
                                             xb[:, k * seg:(k + 1) * seg],
                                             start=(k == 0), stop=(k == 7))
                        cw = CW[nm]
                        c0 = cw[:, j * 4 + 0:j * 4 + 1]
                        c1 = cw[:, j * 4 + 1:j * 4 + 2]
                        c2 = cw[:, j * 4 + 2:j * 4 + 3]
                        c3 = cw[:, j * 4 + 3:j * 4 + 4]
                        # taps: conv[t] = sum_i cw[i]*pre[t-3+i]; ps col (t-t0+4)
                        m0 = cvp.tile([128, TB], F32, tag="m0", name="m0")
                        nc.scalar.activation(m0[:], ps[:, 1:TB + 1], ACTF.Copy,
                                             scale=c0)
                        m1 = cvp.tile([128, TB], F32, tag="m1", name="m1")
                        nc.scalar.activation(m1[:], ps[:, 2:TB + 2], ACTF.Copy,
                                             scale=c1)
                        s2 = cvp.tile([128, TB], F32, tag="s2", name="s2")
                        nc.vector.scalar_tensor_tensor(s2[:], ps[:, 3:TB + 3], c2,
                                                       m0[:], ALU.mult, ALU.add)
                        s3 = cvp.tile([128, TB], F32, tag="s3", name="s3")
                        nc.vector.scalar_tensor_tensor(s3[:], ps[:, 4:TB + 4], c3,
                                                       m1[:], ALU.mult, ALU.add)
                        cv_ = cvp.tile([128, TB], F32, tag="cv", name="cv")
                        nc.gpsimd.tensor_tensor(cv_[:], s2[:], s3[:], ALU.add)
                        st = cvp.tile([128, TB], F32R, tag="st", name="st")
                        if sim_safe:  # CoreSim lacks Silu; HW has it
                            sg = cvp.tile([128, TB], F32, tag="sg", name="sg")
                            nc.scalar.activation(sg[:], cv_[:], ACTF.Sigmoid)
                            nc.gpsimd.tensor_tensor(st[:], cv_[:], sg[:], ALU.mult)
                        else:
                            nc.scalar.activation(st[:], cv_[:], ACTF.Silu)
                        nc.sync.dma_start(
                            TGT[nm][j * 128:(j + 1) * 128, t0:t0 + TB], st[:])

        # ================= chunk stage: delta rule =================
        ckx = top.enter_context(ExitStack())
        work = ckx.enter_context(tc.tile_pool(name="work", bufs=3))
        spool = ckx.enter_context(tc.tile_pool(name="spool", bufs=2))
        ohp = ckx.enter_context(tc.tile_pool(name="ohp", bufs=2))
        wop = ckx.enter_context(tc.tile_pool(name="wop", bufs=1))
        qkt = ckx.enter_context(tc.tile_pool(name="qkt", bufs=4))

        WO = [wop.tile([128, D], F32R, tag=f"wo{k}", name=f"wo{k}") for k in range(4)]
        for k in range(4):
            nc.sync.dma_start(WO[k][:], wo[k * 128:(k + 1) * 128, :])

        ydr = dram.tile([T, D], F32, tag="ydr", name="ydr")

        S = {}
        for h in range(2):
            S[h] = [spool.tile([128, DH], F32R, tag=f"S{h}{i}", name=f"S{h}{i}")
                    for i in range(2)]
            for i in range(2):
                nc.sync.dma_start(S[h][i][:], zs_in[:])

        n_rs = 4 if n_ck % 4 == 0 else 2
        rs_every = n_ck // n_rs
        TBL = T // n_rs          # t-rows per RS block
        HALF = TBL // 2          # rows this core owns per block
        yhb = [dram.tile([HALF, D], F32, tag=f"yhb{rb}", name=f"yhb{rb}")
               for rb in range(n_rs)]

        LNG = const.tile([128, D], F32, tag="lng", name="lng")
        nc.sync.dma_start(LNG[:], lng[:])
        LNB = const.tile([128, D], F32, tag="lnb", name="lnb")
        nc.sync.dma_start(LNB[:], lnb[:])
        lnp = ckx.enter_context(tc.tile_pool(name="lnp", bufs=2))

        def prep(h, c):
            cc = slice(c * CK, (c + 1) * CK)
            r0 = 256 * h
            # stream q/k/v chunk (d-major, two d-tiles side by side; 1 DMA each)
            qc = qkt.tile([128, 256], F32R, tag="qc", name="qc")
            kc = qkt.tile([128, 256], F32R, tag="kc", name="kc")
            vc = qkt.tile([128, 256], F32R, tag="vc", name="vc")
            for t_, dr_ in ((qc, qdr), (kc, kdr), (vc, vdr)):
                nc.sync.dma_start(
                    t_[:].rearrange("p (i t) -> p i t", i=2),
                    dr_[r0:r0 + 256, cc].rearrange("(i p) t -> p i t", p=128))
            brow = BT[h][0:1, cc]

            # -- l2 norm sums: SQ layout [q-d0 | k-d0 | q-d1 | k-d1] --
            SQ = work.tile([128, 512], F32R, tag="SQ", name="SQ")
            for i in range(2):
                nc.scalar.activation(SQ[:, i * 256:i * 256 + 128],
                                     qc[:, i * 128:(i + 1) * 128], ACTF.Square)
                nc.scalar.activation(SQ[:, i * 256 + 128:(i + 1) * 256],
                                     kc[:, i * 128:(i + 1) * 128], ACTF.Square)
            psn = ps_small()
            for i in range(2):
                nc.tensor.matmul(psn[0:1, 0:256], ONESR[:, 0:1],
                                 SQ[:, i * 256:(i + 1) * 256],
                                 start=(i == 0), stop=(i == 1))
            sqr = work.tile([1, 256], F32, tag="sqr", name="sqr")
            nc.scalar.activation(sqr[:], psn[0:1, 0:256], ACTF.Sqrt,
                                 bias=EPS1[0:1, 0:1])
            R3 = work.tile([1, 384], F32R, tag="R3", name="R3")
            nc.vector.reciprocal(R3[0:1, 0:256], sqr[:])
            nc.vector.tensor_tensor(R3[0:1, 256:384], R3[0:1, 128:256], brow,
                                    ALU.mult)
            psbc = ps_small()
            nc.tensor.transpose(psbc[0:128, 0:1], brow, MUI[0:1, 0:1])
            bcol = work.tile([128, 1], F32, tag="bcol", name="bcol", bufs=4)
            nc.scalar.copy(bcol[:], psbc[0:128, 0:1])
            psbr = ps_small()
            nc.tensor.matmul(psbr[0:128, 0:384], ONESR[0:1, :], R3[:],
                             start=True, stop=True)
            QhT = work.tile([128, 256], F32R, tag="QhT", name="QhT", bufs=4)
            KhT = work.tile([128, 256], F32R, tag="KhT", name="KhT", bufs=4)
            KbT = work.tile([128, 256], F32R, tag="KbT", name="KbT", bufs=4)
            for i in range(2):
                sl = slice(i * 128, (i + 1) * 128)
                nc.vector.tensor_tensor(QhT[:, sl], qc[:, sl], psbr[:, 0:128],
                                        ALU.mult)
                nc.vector.tensor_tensor(KhT[:, sl], kc[:, sl], psbr[:, 128:256],
                                        ALU.mult)
                nc.vector.tensor_tensor(KbT[:, sl], kc[:, sl], psbr[:, 256:384],
                                        ALU.mult)

            # -- Gram + masks: N = -tril(Kb K^T,-1), NT = -triu(K Kb^T,1) --
            psg = ps_pg()
            for i in range(2):
                sl = slice(i * 128, (i + 1) * 128)
                nc.tensor.matmul(psg[:], KbT[:, sl], KhT[:, sl],
                                 start=(i == 0), stop=(i == 1))
            N = work.tile([128, 128], F32R, tag="N", name="N")
            nc.vector.tensor_tensor(N[:], psg[:], MLO[:], ALU.mult)
            psgt = ps_pg()
            for i in range(2):
                sl = slice(i * 128, (i + 1) * 128)
                nc.tensor.matmul(psgt[:], KhT[:, sl], KbT[:, sl],
                                 start=(i == 0), stop=(i == 1))
            NT = work.tile([128, 128], F32R, tag="NT", name="NT")
            nc.vector.tensor_tensor(NT[:], psgt[:], MUP[:], ALU.mult)

            # -- truncated inverse: T'^T=(I-A^T)(I+(A^T)^2)(I+(A^T)^4)(I+(A^T)^8)
            pp = ps_pg()
            nc.tensor.matmul(pp[:], NT[:], N[:], start=True, stop=True)   # A^2
            P2 = work.tile([128, 128], F32R, tag="P2", name="P2")
            nc.scalar.copy(P2[:], pp[:])
            ppt = ps_pg()
            nc.tensor.matmul(ppt[:], N[:], NT[:], start=True, stop=True)  # (A^T)^2
            P2T = work.tile([128, 128], F32R, tag="P2T", name="P2T")
            nc.vector.tensor_copy(P2T[:], ppt[:])
            pp4 = ps_pg()
            nc.tensor.matmul(pp4[:], P2T[:], P2[:], start=True, stop=True)  # A^4
            P4 = work.tile([128, 128], F32R, tag="P4", name="P4")
            nc.scalar.copy(P4[:], pp4[:])
            pp4t = ps_pg()
            nc.tensor.matmul(pp4t[:], P2[:], P2T[:], start=True, stop=True)
            P4T = work.tile([128, 128], F32R, tag="P4T", name="P4T")
            nc.vector.tensor_copy(P4T[:], pp4t[:])
            pp8t = ps_pg()
            nc.tensor.matmul(pp8t[:], P4[:], P4T[:], start=True, stop=True)
            R = work.tile([128, 128], F32R, tag="Rch", name="Rch")
            nc.vector.tensor_tensor(R[:], IDENT[:], pp8t[:], ALU.add)
            for P_ in (P4, P2):
                pst = ps_pg()
                nc.tensor.matmul(pst[:], P_[:], R[:], start=True, stop=True)
                R2 = work.tile([128, 128], F32R, tag="Rch", name="Rch")
                nc.vector.tensor_tensor(R2[:], R[:], pst[:], ALU.add)
                R = R2
            pst = ps_pg()
            nc.tensor.matmul(pst[:], N[:], R[:], start=True, stop=True)
            TT = work.tile([128, 128], F32R, tag="TT", name="TT", bufs=4)
            nc.vector.scalar_tensor_tensor(TT[:], R[:], 1.0, pst[:], ALU.mult,
                                           ALU.add)

            # -- beta*V (t-major) --
            vw = ps_med()
            for i in range(2):
                nc.tensor.transpose(vw[:, i * 128:(i + 1) * 128].bitcast(F32R),
                                    vc[:, i * 128:(i + 1) * 128], IDENT[:])
            Vtb = work.tile([128, 256], F32R, tag="Vtb", name="Vtb", bufs=4)
            nc.scalar.activation(Vtb[:], vw[:, 0:256], ACTF.Copy, scale=bcol[:])

            # -- MT = triu(K Q^T) incl diag --
            psmt = ps_pg()
            for i in range(2):
                sl = slice(i * 128, (i + 1) * 128)
                nc.tensor.matmul(psmt[:], KhT[:, sl], QhT[:, sl],
                                 start=(i == 0), stop=(i == 1))
            MT = work.tile([128, 128], F32R, tag="MT", name="MT", bufs=4)
            nc.vector.tensor_tensor(MT[:], psmt[:], MUI[:], ALU.mult)

            # -- K t-major (for the S update) --
            pskt = ps_med()
            for i in range(2):
                nc.tensor.transpose(pskt[:, i * 128:(i + 1) * 128].bitcast(F32R),
                                    KhT[:, i * 128:(i + 1) * 128], IDENT[:])
            Kh = work.tile([128, 256], F32R, tag="Kh", name="Kh", bufs=4)
            nc.scalar.copy(Kh[:], pskt[:, 0:256])

            return dict(QhT=QhT, KhT=KhT, KbT=KbT, TT=TT, Vtb=Vtb, MT=MT, Kh=Kh)

        def spart(h, c, Pd):
            QhT, KbT, TT = Pd["QhT"], Pd["KbT"], Pd["TT"]
            Vtb, MT, Kh = Pd["Vtb"], Pd["MT"], Pd["Kh"]

            # W2b = beta*V - (beta K) S
            psw = ps_med()
            for i in range(2):
                nc.tensor.matmul(psw[:, 0:256], KbT[:, i * 128:(i + 1) * 128],
                                 S[h][i][:], start=(i == 0), stop=(i == 1))
            W2b = work.tile([128, 256], F32R, tag="W2b", name="W2b")
            nc.vector.tensor_tensor(W2b[:], Vtb[:], psw[:, 0:256], ALU.subtract)
            # U = T' W2b
            nc.tensor.matmul(psw[:, 256:512], TT[:], W2b[:], start=True, stop=True)
            U = work.tile([128, 256], F32R, tag="U", name="U")
            nc.scalar.copy(U[:], psw[:, 256:512])

            # O (t-major) = Q S + M U; RMS stats via ACT accum
            pso = ps_med()
            nc.tensor.matmul(pso[:, 0:256], QhT[:, 0:128], S[h][0][:],
                             start=True, stop=False)
            nc.tensor.matmul(pso[:, 0:256], QhT[:, 128:256], S[h][1][:],
                             start=False, stop=False)
            nc.tensor.matmul(pso[:, 0:256], MT[:], U[:], start=False, stop=True)
            sqw = work.tile([128, 256], F32, tag="sqw", name="sqw")
            sso = work.tile([128, 1], F32, tag="sso", name="sso")
            nc.scalar.activation(sqw[:], pso[:, 0:256], ACTF.Square,
                                 accum_out=sso[:])
            sdo = work.tile([128, 1], F32, tag="sdo", name="sdo")
            nc.scalar.activation(sdo[:], sso[:], ACTF.Sqrt, bias=EPSL[:],
                                 scale=1.0 / DH)
            rco = work.tile([128, 1], F32, tag="rco", name="rco")
            nc.vector.reciprocal(rco[:], sdo[:])
            Ohn = work.tile([128, 256], F32R, tag="Ohn", name="Ohn")
            nc.vector.scalar_tensor_tensor(Ohn[:], pso[:, 0:256], rco[:], GRMSB[:],
                                           ALU.mult, ALU.mult)
            for i in range(2):
                nc.tensor.transpose(
                    pso[:, 256 + i * 128:256 + (i + 1) * 128].bitcast(F32R),
                    Ohn[:, i * 128:(i + 1) * 128], IDENT[:])
            OhT = ohp.tile([128, 256], F32R, tag=f"OhT{h}", name=f"OhT{h}")
            nc.scalar.copy(OhT[:], pso[:, 256:512])

            # S += K^T U
            ktds = ps_med()
            for i in range(2):
                reg = slice(0, 256) if i == 0 else slice(256, 512)
                nc.tensor.matmul(ktds[:, reg], Kh[:, i * 128:(i + 1) * 128], U[:],
                                 start=True, stop=True)
                Snew = spool.tile([128, DH], F32R, tag=f"S{h}{i}", name=f"S{h}{i}")
                nc.vector.tensor_tensor(Snew[:], S[h][i][:], ktds[:, reg], ALU.add)
                S[h][i] = Snew
            return OhT

        def ln_rows(src_ap, dst_rows, nrows):
            yr_in = lnp.tile([128, D], F32, tag="yr_in", name="yr_in")
            nc.sync.dma_start(yr_in[0:nrows, :], src_ap)
            xr = lnp.tile([128, D], F32, tag="xr", name="xr")
            nc.sync.dma_start(xr[0:nrows, :], xres[dst_rows, :])
            yr = lnp.tile([128, D], F32, tag="yr", name="yr")
            nc.gpsimd.tensor_tensor(yr[0:nrows, :], yr_in[0:nrows, :],
                                    xr[0:nrows, :], ALU.add)
            waste = lnp.tile([128, D], F32, tag="waste", name="waste", bufs=1)
            srow = lnp.tile([128, 1], F32, tag="srow", name="srow")
            nc.scalar.activation(waste[0:nrows, :], yr[0:nrows, :], ACTF.Identity,
                                 accum_out=srow[0:nrows, :])
            mneg = lnp.tile([128, 1], F32, tag="mneg", name="mneg")
            nc.scalar.mul(mneg[0:nrows, :], srow[0:nrows, :], -1.0 / D)
            yc = lnp.tile([128, D], F32, tag="yc", name="yc")
            nc.scalar.activation(yc[0:nrows, :], yr[0:nrows, :], ACTF.Identity,
                                 bias=mneg[0:nrows, :])
            ssq = lnp.tile([128, 1], F32, tag="ssq", name="ssq")
            nc.scalar.activation(waste[0:nrows, :], yc[0:nrows, :], ACTF.Square,
                                 accum_out=ssq[0:nrows, :])
            sd = lnp.tile([128, 1], F32, tag="sd", name="sd")
            nc.scalar.activation(sd[0:nrows, :], ssq[0:nrows, :], ACTF.Sqrt,
                                 bias=EPSL[0:nrows, :], scale=1.0 / D)
            rcol = lnp.tile([128, 1], F32, tag="rcol", name="rcol")
            nc.vector.reciprocal(rcol[0:nrows, :], sd[0:nrows, :])
            yn = lnp.tile([128, D], F32, tag="yn", name="yn")
            nc.vector.scalar_tensor_tensor(yn[0:nrows, :], yc[0:nrows, :],
                                           rcol[0:nrows, :], LNG[0:nrows, :],
                                           ALU.mult, ALU.mult)
            yfin = lnp.tile([128, D], F32, tag="yfin", name="yfin")
            nc.vector.tensor_tensor(yfin[0:nrows, :], yn[0:nrows, :],
                                    LNB[0:nrows, :], ALU.add)
            nc.sync.dma_start(y_out[dst_rows, :], yfin[0:nrows, :])

        Pmap = {}
        for h in range(2):
            Pmap[(h, 0)] = prep(h, 0)
        for c in range(n_ck):
            for h in range(2):
                if c + 1 < n_ck:
                    Pmap[(h, c + 1)] = prep(h, c + 1)
            oht = [spart(h, c, Pmap.pop((h, c))) for h in range(2)]
            # -- partial y = o @ Wo for this chunk --
            for n in range(2):
                psy = ps_pay()
                for kk in range(4):
                    h, i = kk // 2, kk % 2
                    nc.tensor.matmul(psy[:], oht[h][:, i * 128:(i + 1) * 128],
                                     WO[kk][:, n * 512:(n + 1) * 512],
                                     start=(kk == 0), stop=(kk == 3))
                ysb = work.tile([128, 512], F32, tag="ysb", name="ysb")
                if n == 0:
                    nc.scalar.copy(ysb[:], psy[:])
                else:
                    nc.vector.tensor_copy(ysb[:], psy[:])
                nc.sync.dma_start(ydr[c * CK:(c + 1) * CK, n * 512:(n + 1) * 512],
                                  ysb[:])
            # -- overlapped ReduceScatter + LayerNorm per row-block --
            if (c + 1) % rs_every == 0:
                rb = (c + 1) // rs_every - 1
                nc.gpsimd.collective_compute(
                    "ReduceScatter", ALU.add,
                    replica_groups=[[0, 1], [2, 3], [4, 5], [6, 7]],
                    ins=[ydr[rb * TBL:(rb + 1) * TBL, :]], outs=[yhb[rb].opt()],
                )
                for r0 in range(0, HALF, 128):
                    nr = min(128, HALF - r0)
                    ln_rows(yhb[rb][r0:r0 + nr, :],
                            slice(rb * HALF + r0, rb * HALF + r0 + nr), nr)

    nc.compile()
    return nc


def _shard(inputs, T=T_FULL):
    x = np.ascontiguousarray(np.asarray(inputs["x"]), dtype=np.float32)
    f32 = lambda a: np.ascontiguousarray(np.asarray(a), dtype=np.float32)
    Wq, Wk, Wv = inputs["Wq"], inputs["Wk"], inputs["Wv"]
    Wb, Wo = inputs["Wb"], inputs["Wo"]
    cq, ck, cv = inputs["conv_q"], inputs["conv_k"], inputs["conv_v"]
    g_rms, ln_g, ln_b = inputs["g_rms"], inputs["ln_g"], inputs["ln_b"]
    TH = T // 2

    ident = np.eye(128, dtype=np.float32)
    ii, jj = np.indices((128, 128))
    mlo = -(jj < ii).astype(np.float32)
    mup = -(jj > ii).astype(np.float32)
    mui = (jj >= ii).astype(np.float32)

    in_maps = []
    for c in range(N_CORES):
        b, g = c // 2, c % 2
        gs = slice(g * DG, (g + 1) * DG)
        in_maps.append({
            "xt": f32(np.concatenate([np.zeros((D, 4), np.float32),
                                      x[b, :T].T], axis=1)),
            "xres": f32(x[b, :T][_rows_idx(T)[g]]),
            "wq": f32(np.asarray(Wq)[:, gs]), "wk": f32(np.asarray(Wk)[:, gs]),
            "wv": f32(np.asarray(Wv)[:, gs]),
            "wb2": f32(np.asarray(Wb)[:, 2 * g:2 * g + 2]),
            "cqT": f32(np.asarray(cq)[:, gs].T), "ckT": f32(np.asarray(ck)[:, gs].T),
            "cvT": f32(np.asarray(cv)[:, gs].T),
            "grmsb": f32(np.tile(np.asarray(g_rms)[None, :], (128, 1))),
            "wo": f32(np.asarray(Wo)[gs, :]),
            "lng": f32(np.tile(np.asarray(ln_g)[None, :], (128, 1))),
            "lnb": f32(np.tile(np.asarray(ln_b)[None, :], (128, 1))),
            "ident": ident, "ones": np.ones((128, 128), np.float32),
            "zs": np.zeros((128, DH), np.float32),
            "mlo": mlo, "mup": mup, "mui": mui,
        })
    return in_maps


def kernel(**inputs):
    from concourse.bass_utils import run_bass_kernel_spmd
    T = T_FULL
    if "nc" not in _cache:
        _cache["nc"] = _build(T)
    nc = _cache["nc"]
    in_maps = _shard(inputs, T)
    res = run_bass_kernel_spmd(nc, in_maps, core_ids=list(range(N_CORES)))
    out = np.empty((B, T, D), dtype=np.float32)
    ridx = _rows_idx(T)
    for c in range(N_CORES):
        b, g = c // 2, c % 2
        out[b, ridx[g]] = res.results[c]["y_out"]
    return out
